# revision 36
# baseline (speedup 1.0000x reference)
"""
Bass/Trainium2 kernel for nn_BottleneckShared (moe_routing).

Computation (per sample b):
    rw   = sigmoid(mean_hw(x) @ router_w.T + router_b)          # [E]
    Wk_b = sum_e rw[e] * wk[e]            (k = 1,2,3)           # per-sample conv kernels
    out  = relu(bn3(conv3(relu(bn2(conv2(relu(bn1(conv1(x)))))))) + x)

Sharding: data-parallel over batch. 64 samples -> 8 NeuronCores x 8 samples.

Key device-side design (v2, PE-minimal):
 - PE matmul cost = out-free-size x cycles/row; fp8e4+DoubleRow runs at
   0.5 cycles/row with TWO independent 128-contraction planes per
   instruction (4x fp16 throughput).
 - The per-sample weight combine exploits sigmoid(z) = 0.5 + 0.5*tanh(z/2):
       W_c = 0.5*sum_e W_e  +  sum_e (0.5*tanh(z_e/2)) * W_e
   The mean term is sample-independent: host precomputes it as an fp8
   hi + (lo*16) pair (combined on PE with an [I | I/16] fp8 lhsT, one
   DoubleRow matmul). The delta term is ~2% of the mean, so single fp8
   precision suffices: expert pairs go in one DoubleRow matmul each with
   lhsT = [tanh_e*I | tanh_e'*I] (fp8, generated per-sample on DVE/Pool).
   Per 512-col chunk: 5 DoubleRow matmuls vs 8 fp16 matmuls -> 3.2x.
 - Router pooling runs on the PE (28 ap_size=8 matmuls: lhsT = x pixel
   chunks, rhs = router weights; psum-accumulated), then a ones-broadcast
   matmul reduces the 112 partial partitions and broadcasts z to all 128
   partitions; ACT applies tanh(z/2) via activation scale=0.5.
 - Convs stay fp16 (x and activations need >fp8 precision; precision
   hi/lo splitting costs as much as fp16).
 - Residual + relu run off-PE: conv3 psum drains via ACT Identity(+bn3
   bias) or DVE STT(+bias+x), then DVE tensor_scalar max(.,0) -> fp16 out.
 - All PSUM->SBUF drains are spread across ACT/DVE/Pool so every
   non-tensor engine stays below the PE's ~7.9us/sample.
"""

import sys

import ml_dtypes
import numpy as np

sys.path.insert(0, "/opt/trn_rl_repo")

import concourse.bacc as bacc
import concourse.bass as bass
import concourse.mybir as mybir
import concourse.tile as tile
from concourse import bass_utils

EPS = 1e-5

B = 64          # global batch
NCORES = 8
BS = B // NCORES  # samples per core
E = 8           # experts
CIN = 512
WID = 128       # bottleneck width
COUT = 512
H = 28
P = H * H       # 784 pixels
NCH = 392       # pixels per conv output chunk (14 rows)
PC = 112        # router pixel chunk (784 = 7*112)

F8 = mybir.dt.float8e4
F16 = mybir.dt.float16
F32 = mybir.dt.float32
NP8 = ml_dtypes.float8_e4m3

# combine chunks: (dst, dst_off, width, bank_name, mean_off)
CHUNKS = [
    ("w1c", 0, 512, "db1", 0),
    ("w2c", 0, 512, "db2a", 1024),
    ("w2c", 512, 512, "db2b", 2048),
    ("w2c", 1024, 128, "db2c", 3072),
    ("w3c", 0, 512, "db3", 3328),
]
MEANW = 4352  # 2 * 2176


def _r2(ap):
    return ap.rearrange("p (two n) -> p two n", two=2)


def build_program():
    nc = bacc.Bacc("TRN2", target_bir_lowering=False, debug=False)

    # ---- DRAM I/O (per-core shapes) ----
    x_d = nc.dram_tensor("x", [BS, 4, 128, P], F16, kind="ExternalInput")
    db1_d = nc.dram_tensor("db1", [128, E * 512], F8, kind="ExternalInput")
    db2a_d = nc.dram_tensor("db2a", [128, E * 512], F8, kind="ExternalInput")
    db2b_d = nc.dram_tensor("db2b", [128, E * 512], F8, kind="ExternalInput")
    db2c_d = nc.dram_tensor("db2c", [128, E * 128], F8, kind="ExternalInput")
    db3_d = nc.dram_tensor("db3", [128, E * 512], F8, kind="ExternalInput")
    dmean_d = nc.dram_tensor("dmean", [128, MEANW], F8, kind="ExternalInput")
    idpair_d = nc.dram_tensor("idpair", [128, 256], F8, kind="ExternalInput")
    cc_d = nc.dram_tensor("cc", [128, 169], F16, kind="ExternalInput")
    biasp_d = nc.dram_tensor("biasp", [128, 6], F32, kind="ExternalInput")
    out_d = nc.dram_tensor("out", [BS, 4, 128, P], F16, kind="ExternalOutput")

    Relu = mybir.ActivationFunctionType.Relu
    Tanh = mybir.ActivationFunctionType.Tanh
    Ident = mybir.ActivationFunctionType.Identity
    Copy = mybir.ActivationFunctionType.Copy
    ADD = mybir.AluOpType.add
    DR = mybir.MatmulPerfMode.DoubleRow
    mm = nc.tensor.matmul

    with tile.TileContext(nc) as tc:
        with (
            tc.tile_pool(name="const", bufs=1) as constp,
            tc.tile_pool(name="xin", bufs=6) as xp,
            tc.tile_pool(name="xsplit", bufs=1) as xsp,
            tc.tile_pool(name="ids", bufs=3) as idsp,
            tc.tile_pool(name="comb", bufs=2) as combp,
            tc.tile_pool(name="act", bufs=2) as actp,
            tc.tile_pool(name="small", bufs=3) as smallp,
            tc.tile_pool(name="t3", bufs=4) as t3p,
            tc.tile_pool(name="rwb", bufs=3) as rwbp,
            tc.tile_pool(name="pscomb", bufs=2, space=bass.MemorySpace.PSUM) as pscombp,
            tc.tile_pool(name="psc12", bufs=3, space=bass.MemorySpace.PSUM) as psc12p,
            tc.tile_pool(name="psc3", bufs=2, space=bass.MemorySpace.PSUM) as psc3p,
            tc.tile_pool(name="psr", bufs=1, space=bass.MemorySpace.PSUM) as psrp,
        ):
            # ---- persistent constants ----
            db1 = constp.tile([128, E * 512], F8)
            db2a = constp.tile([128, E * 512], F8)
            db2b = constp.tile([128, E * 512], F8)
            db2c = constp.tile([128, E * 128], F8)
            db3 = constp.tile([128, E * 512], F8)
            dmean = constp.tile([128, MEANW], F8)
            idpair = constp.tile([128, 256], F8)
            cc = constp.tile([128, 169], F16)
            rwt = cc[:, 0:32]
            ident = cc[:, 32:160]
            onescol = cc[:, 160:161]
            rbrow = cc[0:1, 161:169]
            biasp = constp.tile([128, 6], F32)
            bias1 = biasp[:, 0:1]
            bias2 = biasp[:, 1:2]
            bias3 = biasp[:, 2:6]
            # persistent padded conv2 inputs; border zeroed once
            mid1s = [
                constp.tile([128, 30, 30], F16, name=f"mid1_{i}") for i in range(2)
            ]

            banks = {"db1": db1, "db2a": db2a, "db2b": db2b, "db2c": db2c, "db3": db3}

            # DMA issue order = device service order.
            xs_l, ids_l, tanh_l = [], [], []
            for s in range(BS):
                big = xp.tile([128, 4 * P], F16, tag="xs", name=f"xs{s}")
                xs_l.append([big[:, t * P : (t + 1) * P] for t in range(4)])

            def load_x(s):
                nc.sync.dma_start(
                    xs_l[s][0].tensor[:, :], x_d[s].transpose([1, 0, 2])
                )

            nc.sync.dma_start(cc[:], cc_d[:])
            load_x(0)
            nc.sync.dma_start(db1[:], db1_d[:])
            nc.sync.dma_start(idpair[:], idpair_d[:])
            nc.sync.dma_start(dmean[:, :2048], dmean_d[:, :2048])
            nc.sync.dma_start(biasp[:], biasp_d[:])
            nc.sync.dma_start(db2a[:], db2a_d[:])
            load_x(1)
            nc.sync.dma_start(db2b[:], db2b_d[:])
            nc.sync.dma_start(dmean[:, 2048:], dmean_d[:, 2048:])
            nc.sync.dma_start(db2c[:], db2c_d[:])
            nc.sync.dma_start(db3[:], db3_d[:])
            for s in range(2, BS):
                load_x(s)

            for m1 in mid1s:
                nc.gpsimd.memset(m1[:], 0.0)

            # ================= router ======================================
            def emit_router(s):
                xs = xs_l[s]
                psr1 = psrp.tile([128, 8], F32, tag="psr", name=f"psr1_{s}")
                idx = 0
                for t in range(4):
                    for pc in range(7):
                        mm(
                            psr1[:PC, :],
                            xs[t][:, pc * PC : (pc + 1) * PC],
                            rwt[:, t * 8 : (t + 1) * 8],
                            start=(idx == 0),
                            stop=(idx == 27),
                        )
                        idx += 1
                r1 = smallp.tile([PC, 8], F16, tag="r1")
                nc.vector.tensor_copy(r1[:], psr1[:PC, :])
                psr2 = psrp.tile([128, 8], F32, tag="psr", name=f"psr2_{s}")
                mm(
                    psr2[:],
                    onescol[:PC, 0:1].broadcast_to([PC, 128]),
                    r1[:],
                    start=True,
                    stop=False,
                )
                mm(
                    psr2[:],
                    onescol[0:1, 0:1].broadcast_to([1, 128]),
                    rbrow,
                    start=False,
                    stop=True,
                )
                tanhv = rwbp.tile([128, 8], F32, tag="tanhv", name=f"tanhv{s}")
                nc.scalar.activation(tanhv[:], psr2[:], Tanh, scale=0.5)
                tanh_l.append(tanhv)
                ids8 = idsp.tile([128, E * 128], F8, tag="ids8", name=f"ids8{s}")
                for e in range(E):
                    eng = nc.vector if e % 2 == 0 else nc.gpsimd
                    eng.tensor_scalar_mul(
                        ids8[:, e * 128 : (e + 1) * 128], ident, tanhv[:, e : e + 1]
                    )
                ids_l.append(ids8)

            # ---- combine all chunks on PE (fp8 DoubleRow), 1 sample ahead ----
            w_l = []

            def alloc_w(s):
                w1c = combp.tile([128, 512], F16, tag="w1c", name=f"w1c{s}")
                w2c = combp.tile([128, 1152], F16, tag="w2c", name=f"w2c{s}")
                w3c = combp.tile([128, 512], F16, tag="w3c", name=f"w3c{s}")
                w_l.append((w1c, w2c, w3c))

            def emit_combine_chunk(s, i):
                ids8 = ids_l[s]
                w1c, w2c, w3c = w_l[s]
                dsts = {"w1c": w1c, "w2c": w2c, "w3c": w3c}
                dname, d0, wid, bname, moff = CHUNKS[i]
                bank = banks[bname]
                psc = pscombp.tile([128, 512], F32, tag="psc")
                mm(
                    psc[:, :wid],
                    _r2(idpair[:]),
                    _r2(dmean[:, moff : moff + 2 * wid]),
                    start=True,
                    stop=False,
                    perf_mode=DR,
                )
                for j in range(4):
                    mm(
                        psc[:, :wid],
                        _r2(ids8[:, j * 256 : (j + 1) * 256]),
                        _r2(bank[:, j * 2 * wid : (j + 1) * 2 * wid]),
                        start=False,
                        stop=(j == 3),
                        perf_mode=DR,
                    )
                dst = dsts[dname][:, d0 : d0 + wid]
                # drains: GPSIMD cannot read PSUM -> alternate DVE / ACT
                if i in (0, 2):
                    nc.vector.tensor_scalar_add(dst, psc[:, :wid], 0.0)
                else:
                    nc.scalar.activation(dst, psc[:, :wid], Copy)

            def emit_conv3_half(s, m, c, ofull, path_a):
                xs = xs_l[s]
                w3c = w_l[s][2]
                out2 = out2_l[s]
                ps3 = psc3p.tile([128, 14, 28], F32, tag="c3ps")
                mm(
                    ps3[:],
                    w3c[:, m * 128 : (m + 1) * 128],
                    out2[:, c * NCH : (c + 1) * NCH],
                    start=True,
                    stop=True,
                )
                ps3f = ps3[:].rearrange("p a b -> p (a b)")
                xch = xs[m][:, c * NCH : (c + 1) * NCH]
                dst = ofull[:, m * P + c * NCH : m * P + (c + 1) * NCH]
                u = t3p.tile([128, NCH], F16, tag="u3")
                if path_a:
                    # ACT drains bn3+bias, DVE adds residual, Pool relus
                    t = t3p.tile([128, NCH], F16, tag="t3")
                    nc.scalar.activation(
                        t[:], ps3f, Ident, bias=bias3[:, m : m + 1]
                    )
                    nc.vector.tensor_tensor(u[:], t[:], xch, op=ADD)
                    nc.vector.tensor_scalar_max(dst, u[:], 0.0)
                else:
                    # single DVE pass: (psum + bias3) + x; relu on Pool
                    nc.vector.scalar_tensor_tensor(
                        u[:], ps3f, bias3[:, m : m + 1], xch,
                        op0=ADD, op1=ADD,
                    )
                    nc.gpsimd.tensor_scalar_max(dst, u[:], 0.0)

            def emit_conv3_m(s, m, ofull):
                emit_conv3_half(s, m, 0, ofull, True)
                emit_conv3_half(s, m, 1, ofull, False)
                nc.sync.dma_start(out_d[s, m], ofull[:, m * P : (m + 1) * P])

            emit_router(0)
            alloc_w(0)
            emit_combine_chunk(0, 0)

            # ============ per-sample convs (combine pipelined ahead) ========
            out2_l = {}
            for s in range(BS):
                xs = xs_l[s]
                w1c, w2c, w3c = w_l[s]

                # ---- conv1 (1x1) + bn1 + relu -> padded mid1 [128, 30, 30] ----
                mid1 = mid1s[s % 2]
                for c in range(2):
                    ps1 = psc12p.tile([128, 14, 28], F32, tag="convps")
                    for k in range(4):
                        mm(
                            ps1[:],
                            w1c[:, k * 128 : (k + 1) * 128],
                            xs[k][:, c * NCH : (c + 1) * NCH],
                            start=(k == 0),
                            stop=(k == 3),
                        )
                    nc.scalar.activation(
                        mid1[:, 14 * c + 1 : 14 * c + 15, 1:29],
                        ps1[:],
                        Relu,
                        bias=bias1[:],
                    )

                if s == 0:
                    # stagger sample 0's remaining combine chunks behind the
                    # bank DMAs instead of head-blocking the PE in the prologue
                    emit_combine_chunk(0, 1)
                    emit_combine_chunk(0, 2)
                    emit_combine_chunk(0, 3)

                # ---- conv2 (3x3, pad 1) + bn2 + relu -> out2 [128, 784] ----
                out2 = actp.tile([128, P], F16, tag="out2")
                out2_l[s] = out2
                last = s == BS - 1
                if last:
                    ofull_t = actp.tile([128, 4 * P], F16, tag="ofull",
                                        name="ofull_last")
                for c in range(2):
                    ps2 = psc12p.tile([128, 14, 28], F32, tag="convps")
                    idx = 0
                    for dy in range(3):
                        for dx in range(3):
                            mm(
                                ps2[:],
                                w2c[:, (dy * 3 + dx) * 128 : (dy * 3 + dx + 1) * 128],
                                mid1[:, 14 * c + dy : 14 * c + dy + 14, dx : dx + 28],
                                start=(idx == 0),
                                stop=(idx == 8),
                            )
                            idx += 1
                    nc.scalar.activation(
                        out2[:, c * NCH : (c + 1) * NCH], ps2[:], Relu, bias=bias2[:]
                    )
                    if last:
                        # drain the last sample's conv3 halves as early as
                        # possible: the kernel end is gated by these drains
                        # and by DMA issue time, not by the PE -> spread the
                        # 8 half-DMAs across two descriptor queues
                        for m in range(4):
                            emit_conv3_half(s, m, c, ofull_t, (m + c) % 2 == 0)
                            q = (nc.sync, nc.scalar, nc.gpsimd, nc.scalar)[m]
                            q.dma_start(
                                out_d[s, m][:, c * NCH : (c + 1) * NCH],
                                ofull_t[:, m * P + c * NCH : m * P + (c + 1) * NCH],
                            )

                if s == 0:
                    emit_combine_chunk(0, 4)
                    emit_router(1)
                elif s + 2 < BS:
                    emit_router(s + 2)

                # ---- conv3 + bn3 + residual + relu, interleaved with the
                # next sample's combine so psum drains keep pace with the PE --
                if s + 1 < BS:
                    ofull = actp.tile([128, 4 * P], F16, tag="ofull")
                    alloc_w(s + 1)
                    if s == 0:
                        # give ids8(1) (generated just above) a head start
                        emit_conv3_m(0, 0, ofull)
                        emit_conv3_m(0, 1, ofull)
                        emit_combine_chunk(1, 0)
                        emit_conv3_m(0, 2, ofull)
                        emit_combine_chunk(1, 1)
                        emit_conv3_m(0, 3, ofull)
                        emit_combine_chunk(1, 2)
                        emit_combine_chunk(1, 3)
                        emit_combine_chunk(1, 4)
                        emit_router(2)
                    else:
                        emit_combine_chunk(s + 1, 0)
                        emit_conv3_m(s, 0, ofull)
                        emit_combine_chunk(s + 1, 1)
                        emit_conv3_m(s, 1, ofull)
                        emit_combine_chunk(s + 1, 2)
                        emit_conv3_m(s, 2, ofull)
                        emit_combine_chunk(s + 1, 3)
                        emit_combine_chunk(s + 1, 4)
                        emit_conv3_m(s, 3, ofull)

    nc.compile()
    return nc


_NC_CACHE = None


def _get_program():
    global _NC_CACHE
    if _NC_CACHE is None:
        _NC_CACHE = build_program()
    return _NC_CACHE


def prepare_inputs(
    x, router_w, router_b, w1, w2, w3,
    g1, b1, m1, v1, g2, b2, m2, v2, g3, b3, m3, v3,
):
    """Host-side preprocessing -> per-core in_maps."""
    f = np.float32
    x = np.asarray(x, f)
    router_w = np.asarray(router_w, f)
    router_b = np.asarray(router_b, f)
    w1 = np.asarray(w1, f)
    w2 = np.asarray(w2, f)
    w3 = np.asarray(w3, f)

    s1 = np.asarray(g1, f) / np.sqrt(np.asarray(v1, f) + EPS)
    s2 = np.asarray(g2, f) / np.sqrt(np.asarray(v2, f) + EPS)
    s3 = np.asarray(g3, f) / np.sqrt(np.asarray(v3, f) + EPS)
    bb1 = np.asarray(b1, f) - np.asarray(m1, f) * s1
    bb2 = np.asarray(b2, f) - np.asarray(m2, f) * s2
    bb3 = np.asarray(b3, f) - np.asarray(m3, f) * s3

    # bank1: [E, o=128, i=512] * s1[o] -> rows i%128, cols (e, it, o)
    w1s = w1[:, :, :, 0, 0] * s1[None, :, None]
    bank1 = np.ascontiguousarray(
        w1s.transpose(0, 2, 1).reshape(E, 4, 128, 128).transpose(2, 0, 1, 3)
        .reshape(128, E * 512)
    )
    # bank2: [E, o, ci, dy, dx] * s2[o] -> rows ci, cols (e, tap, o)
    w2s = w2 * s2[None, :, None, None, None]
    b2flat = (
        w2s.transpose(0, 3, 4, 2, 1).reshape(E, 9, 128, 128).transpose(2, 0, 1, 3)
        .reshape(128, E, 1152)
    )
    # bank3: [E, o=512, ci=128] * s3[o] -> rows ci, cols (e, m, o)
    w3s = w3[:, :, :, 0, 0] * s3[None, :, None]
    bank3 = np.ascontiguousarray(
        w3s.transpose(0, 2, 1).transpose(1, 0, 2).reshape(128, E * 512)
    )

    # per-chunk [128, E, wid] views, expert-major delta banks (x0.5) in fp8
    chunks = {
        "db1": bank1.reshape(128, E, 512),
        "db2a": b2flat[:, :, 0:512],
        "db2b": b2flat[:, :, 512:1024],
        "db2c": b2flat[:, :, 1024:1152],
        "db3": bank3.reshape(128, E, 512),
    }
    dbanks = {
        k: np.ascontiguousarray((0.5 * v).reshape(128, -1)).astype(NP8)
        for k, v in chunks.items()
    }
    # mean term: hi fp8 + (residual*16) fp8, chunk-ordered
    mean_parts = []
    for k in ("db1", "db2a", "db2b", "db2c", "db3"):
        M = 0.5 * chunks[k].sum(axis=1)          # [128, wid]
        hi = M.astype(NP8)
        lo = ((M - hi.astype(f)) * 16.0).astype(NP8)
        mean_parts += [hi, lo]
    dmean = np.concatenate(mean_parts, axis=1)
    assert dmean.shape == (128, MEANW)

    idpair = np.zeros((128, 256), NP8)
    idpair[:, 0:128] = np.eye(128, dtype=NP8)
    idpair[:, 128:256] = (np.eye(128, dtype=f) / 16.0).astype(NP8)

    rwt = np.ascontiguousarray(
        (router_w / float(P)).T.reshape(4, 128, E)
    ).astype(np.float16)
    cc = np.zeros((128, 169), np.float16)
    cc[:, 0:32] = rwt.transpose(1, 0, 2).reshape(128, 32)
    cc[:, 32:160] = np.eye(128, dtype=np.float16)
    cc[:, 160] = 1.0
    cc[0, 161:169] = router_b.astype(np.float16)
    biasp = np.zeros((128, 6), f)
    biasp[:, 0] = bb1
    biasp[:, 1] = bb2
    biasp[:, 2:6] = bb3.reshape(4, 128).T

    x16 = x.reshape(B, 4, 128, P).astype(np.float16)

    shared = {
        **dbanks,
        "dmean": dmean,
        "idpair": idpair,
        "cc": cc,
        "biasp": biasp,
    }
    in_maps = []
    for c in range(NCORES):
        m = dict(shared)
        m["x"] = np.ascontiguousarray(x16[c * BS : (c + 1) * BS])
        in_maps.append(m)
    return in_maps


def run(in_maps, trace=False, tmpdir=None):
    nc = _get_program()
    res = bass_utils.run_bass_kernel_spmd(
        nc, in_maps, core_ids=list(range(NCORES)), trace=trace, tmpdir=tmpdir
    )
    outs = [np.asarray(r["out"], np.float32) for r in res.results]
    full = np.concatenate(outs, axis=0).reshape(B, CIN, H, H)
    return full, res


def kernel(**inputs):
    in_maps = prepare_inputs(**inputs)
    full, _ = run(in_maps, trace=False)
    return full


# revision 37
# speedup vs baseline: 1.0074x; 1.0074x over previous
"""
Bass/Trainium2 kernel for nn_BottleneckShared (moe_routing).

Computation (per sample b):
    rw   = sigmoid(mean_hw(x) @ router_w.T + router_b)          # [E]
    Wk_b = sum_e rw[e] * wk[e]            (k = 1,2,3)           # per-sample conv kernels
    out  = relu(bn3(conv3(relu(bn2(conv2(relu(bn1(conv1(x)))))))) + x)

Sharding: data-parallel over batch. 64 samples -> 8 NeuronCores x 8 samples.

Key device-side design (v2, PE-minimal):
 - PE matmul cost = out-free-size x cycles/row; fp8e4+DoubleRow runs at
   0.5 cycles/row with TWO independent 128-contraction planes per
   instruction (4x fp16 throughput).
 - The per-sample weight combine exploits sigmoid(z) = 0.5 + 0.5*tanh(z/2):
       W_c = 0.5*sum_e W_e  +  sum_e (0.5*tanh(z_e/2)) * W_e
   The mean term is sample-independent: host precomputes it as an fp8
   hi + (lo*16) pair (combined on PE with an [I | I/16] fp8 lhsT, one
   DoubleRow matmul). The delta term is ~2% of the mean, so single fp8
   precision suffices: expert pairs go in one DoubleRow matmul each with
   lhsT = [tanh_e*I | tanh_e'*I] (fp8, generated per-sample on DVE/Pool).
   Per 512-col chunk: 5 DoubleRow matmuls vs 8 fp16 matmuls -> 3.2x.
 - Router pooling runs on the PE (28 ap_size=8 matmuls: lhsT = x pixel
   chunks, rhs = router weights; psum-accumulated), then a ones-broadcast
   matmul reduces the 112 partial partitions and broadcasts z to all 128
   partitions; ACT applies tanh(z/2) via activation scale=0.5.
 - Convs stay fp16 (x and activations need >fp8 precision; precision
   hi/lo splitting costs as much as fp16).
 - Residual + relu run off-PE: conv3 psum drains via ACT Identity(+bn3
   bias) or DVE STT(+bias+x), then DVE tensor_scalar max(.,0) -> fp16 out.
 - All PSUM->SBUF drains are spread across ACT/DVE/Pool so every
   non-tensor engine stays below the PE's ~7.9us/sample.
"""

import sys

import ml_dtypes
import numpy as np

sys.path.insert(0, "/opt/trn_rl_repo")

import concourse.bacc as bacc
import concourse.bass as bass
import concourse.mybir as mybir
import concourse.tile as tile
from concourse import bass_utils

EPS = 1e-5

B = 64          # global batch
NCORES = 8
BS = B // NCORES  # samples per core
E = 8           # experts
CIN = 512
WID = 128       # bottleneck width
COUT = 512
H = 28
P = H * H       # 784 pixels
NCH = 392       # pixels per conv output chunk (14 rows)
PC = 112        # router pixel chunk (784 = 7*112)

F8 = mybir.dt.float8e4
F16 = mybir.dt.float16
F32 = mybir.dt.float32
NP8 = ml_dtypes.float8_e4m3

# combine chunks: (dst, dst_off, width, bank_name, mean_off)
CHUNKS = [
    ("w1c", 0, 512, "db1", 0),
    ("w2c", 0, 512, "db2a", 1024),
    ("w2c", 512, 512, "db2b", 2048),
    ("w2c", 1024, 128, "db2c", 3072),
    ("w3c", 0, 512, "db3", 3328),
]
MEANW = 4352  # 2 * 2176


def _r2(ap):
    return ap.rearrange("p (two n) -> p two n", two=2)


def build_program():
    nc = bacc.Bacc("TRN2", target_bir_lowering=False, debug=False)

    # ---- DRAM I/O (per-core shapes) ----
    x_d = nc.dram_tensor("x", [BS, 4, 128, P], F16, kind="ExternalInput")
    db1_d = nc.dram_tensor("db1", [128, E * 512], F8, kind="ExternalInput")
    db2a_d = nc.dram_tensor("db2a", [128, E * 512], F8, kind="ExternalInput")
    db2b_d = nc.dram_tensor("db2b", [128, E * 512], F8, kind="ExternalInput")
    db2c_d = nc.dram_tensor("db2c", [128, E * 128], F8, kind="ExternalInput")
    db3_d = nc.dram_tensor("db3", [128, E * 512], F8, kind="ExternalInput")
    dmean_d = nc.dram_tensor("dmean", [128, MEANW], F8, kind="ExternalInput")
    idpair_d = nc.dram_tensor("idpair", [128, 256], F8, kind="ExternalInput")
    cc_d = nc.dram_tensor("cc", [128, 169], F16, kind="ExternalInput")
    biasp_d = nc.dram_tensor("biasp", [128, 6], F32, kind="ExternalInput")
    out_d = nc.dram_tensor("out", [BS, 4, 128, P], F16, kind="ExternalOutput")

    Relu = mybir.ActivationFunctionType.Relu
    Tanh = mybir.ActivationFunctionType.Tanh
    Ident = mybir.ActivationFunctionType.Identity
    Copy = mybir.ActivationFunctionType.Copy
    ADD = mybir.AluOpType.add
    DR = mybir.MatmulPerfMode.DoubleRow
    mm = nc.tensor.matmul

    with tile.TileContext(nc) as tc:
        with (
            tc.tile_pool(name="const", bufs=1) as constp,
            tc.tile_pool(name="xin", bufs=6) as xp,
            tc.tile_pool(name="xsplit", bufs=1) as xsp,
            tc.tile_pool(name="ids", bufs=3) as idsp,
            tc.tile_pool(name="comb", bufs=2) as combp,
            tc.tile_pool(name="act", bufs=2) as actp,
            tc.tile_pool(name="small", bufs=3) as smallp,
            tc.tile_pool(name="t3", bufs=4) as t3p,
            tc.tile_pool(name="rwb", bufs=3) as rwbp,
            tc.tile_pool(name="pscomb", bufs=2, space=bass.MemorySpace.PSUM) as pscombp,
            tc.tile_pool(name="psc12", bufs=3, space=bass.MemorySpace.PSUM) as psc12p,
            tc.tile_pool(name="psc3", bufs=2, space=bass.MemorySpace.PSUM) as psc3p,
            tc.tile_pool(name="psr", bufs=1, space=bass.MemorySpace.PSUM) as psrp,
        ):
            # ---- persistent constants ----
            db1 = constp.tile([128, E * 512], F8)
            db2a = constp.tile([128, E * 512], F8)
            db2b = constp.tile([128, E * 512], F8)
            db2c = constp.tile([128, E * 128], F8)
            db3 = constp.tile([128, E * 512], F8)
            dmean = constp.tile([128, MEANW], F8)
            idpair = constp.tile([128, 256], F8)
            cc = constp.tile([128, 169], F16)
            rwt = cc[:, 0:32]
            ident = cc[:, 32:160]
            onescol = cc[:, 160:161]
            rbrow = cc[0:1, 161:169]
            biasp = constp.tile([128, 6], F32)
            bias1 = biasp[:, 0:1]
            bias2 = biasp[:, 1:2]
            bias3 = biasp[:, 2:6]
            # persistent padded conv2 inputs; border zeroed once
            mid1s = [
                constp.tile([128, 30, 30], F16, name=f"mid1_{i}") for i in range(2)
            ]

            banks = {"db1": db1, "db2a": db2a, "db2b": db2b, "db2c": db2c, "db3": db3}

            # DMA issue order = device service order.
            xs_l, ids_l, tanh_l = [], [], []
            for s in range(BS):
                big = xp.tile([128, 4 * P], F16, tag="xs", name=f"xs{s}")
                xs_l.append([big[:, t * P : (t + 1) * P] for t in range(4)])

            def load_x(s):
                nc.sync.dma_start(
                    xs_l[s][0].tensor[:, :], x_d[s].transpose([1, 0, 2])
                )

            nc.sync.dma_start(cc[:], cc_d[:])
            # x0 split in halves so the router can start on tiles 0-1 early
            x0t = xs_l[0][0].tensor
            nc.sync.dma_start(
                x0t[:, : 2 * P], x_d[0, 0:2].transpose([1, 0, 2])
            )
            nc.sync.dma_start(
                x0t[:, 2 * P :], x_d[0, 2:4].transpose([1, 0, 2])
            )
            nc.sync.dma_start(idpair[:], idpair_d[:])
            nc.sync.dma_start(dmean[:, :2048], dmean_d[:, :2048])
            nc.sync.dma_start(db1[:], db1_d[:])
            nc.sync.dma_start(biasp[:], biasp_d[:])
            nc.sync.dma_start(db2a[:], db2a_d[:])
            load_x(1)
            nc.sync.dma_start(db2b[:], db2b_d[:])
            nc.sync.dma_start(dmean[:, 2048:], dmean_d[:, 2048:])
            nc.sync.dma_start(db2c[:], db2c_d[:])
            nc.sync.dma_start(db3[:], db3_d[:])
            for s in range(2, BS):
                load_x(s)

            for m1 in mid1s:
                nc.gpsimd.memset(m1[:], 0.0)

            # ================= router ======================================
            def emit_router(s):
                xs = xs_l[s]
                psr1 = psrp.tile([128, 8], F32, tag="psr", name=f"psr1_{s}")
                idx = 0
                for t in range(4):
                    for pc in range(7):
                        mm(
                            psr1[:PC, :],
                            xs[t][:, pc * PC : (pc + 1) * PC],
                            rwt[:, t * 8 : (t + 1) * 8],
                            start=(idx == 0),
                            stop=(idx == 27),
                        )
                        idx += 1
                r1 = smallp.tile([PC, 8], F16, tag="r1")
                nc.vector.tensor_copy(r1[:], psr1[:PC, :])
                psr2 = psrp.tile([128, 8], F32, tag="psr", name=f"psr2_{s}")
                mm(
                    psr2[:],
                    onescol[:PC, 0:1].broadcast_to([PC, 128]),
                    r1[:],
                    start=True,
                    stop=False,
                )
                mm(
                    psr2[:],
                    onescol[0:1, 0:1].broadcast_to([1, 128]),
                    rbrow,
                    start=False,
                    stop=True,
                )
                tanhv = rwbp.tile([128, 8], F32, tag="tanhv", name=f"tanhv{s}")
                nc.scalar.activation(tanhv[:], psr2[:], Tanh, scale=0.5)
                tanh_l.append(tanhv)
                ids8 = idsp.tile([128, E * 128], F8, tag="ids8", name=f"ids8{s}")
                for e in range(E):
                    eng = nc.vector if e % 2 == 0 else nc.gpsimd
                    eng.tensor_scalar_mul(
                        ids8[:, e * 128 : (e + 1) * 128], ident, tanhv[:, e : e + 1]
                    )
                ids_l.append(ids8)

            # ---- combine all chunks on PE (fp8 DoubleRow), 1 sample ahead ----
            w_l = []

            def alloc_w(s):
                w1c = combp.tile([128, 512], F16, tag="w1c", name=f"w1c{s}")
                w2c = combp.tile([128, 1152], F16, tag="w2c", name=f"w2c{s}")
                w3c = combp.tile([128, 512], F16, tag="w3c", name=f"w3c{s}")
                w_l.append((w1c, w2c, w3c))

            def emit_combine_chunk(s, i):
                ids8 = ids_l[s]
                w1c, w2c, w3c = w_l[s]
                dsts = {"w1c": w1c, "w2c": w2c, "w3c": w3c}
                dname, d0, wid, bname, moff = CHUNKS[i]
                bank = banks[bname]
                psc = pscombp.tile([128, 512], F32, tag="psc")
                mm(
                    psc[:, :wid],
                    _r2(idpair[:]),
                    _r2(dmean[:, moff : moff + 2 * wid]),
                    start=True,
                    stop=False,
                    perf_mode=DR,
                )
                for j in range(4):
                    mm(
                        psc[:, :wid],
                        _r2(ids8[:, j * 256 : (j + 1) * 256]),
                        _r2(bank[:, j * 2 * wid : (j + 1) * 2 * wid]),
                        start=False,
                        stop=(j == 3),
                        perf_mode=DR,
                    )
                dst = dsts[dname][:, d0 : d0 + wid]
                # drains: GPSIMD cannot read PSUM -> alternate DVE / ACT
                if i in (0, 2):
                    nc.vector.tensor_scalar_add(dst, psc[:, :wid], 0.0)
                else:
                    nc.scalar.activation(dst, psc[:, :wid], Copy)

            def emit_conv3_half(s, m, c, ofull, path_a):
                xs = xs_l[s]
                w3c = w_l[s][2]
                out2 = out2_l[s]
                ps3 = psc3p.tile([128, 14, 28], F32, tag="c3ps")
                mm(
                    ps3[:],
                    w3c[:, m * 128 : (m + 1) * 128],
                    out2[:, c * NCH : (c + 1) * NCH],
                    start=True,
                    stop=True,
                )
                ps3f = ps3[:].rearrange("p a b -> p (a b)")
                xch = xs[m][:, c * NCH : (c + 1) * NCH]
                dst = ofull[:, m * P + c * NCH : m * P + (c + 1) * NCH]
                u = t3p.tile([128, NCH], F16, tag="u3")
                if path_a:
                    # ACT drains bn3+bias, DVE adds residual, Pool relus
                    t = t3p.tile([128, NCH], F16, tag="t3")
                    nc.scalar.activation(
                        t[:], ps3f, Ident, bias=bias3[:, m : m + 1]
                    )
                    nc.vector.tensor_tensor(u[:], t[:], xch, op=ADD)
                    nc.vector.tensor_scalar_max(dst, u[:], 0.0)
                else:
                    # single DVE pass: (psum + bias3) + x; relu on Pool
                    nc.vector.scalar_tensor_tensor(
                        u[:], ps3f, bias3[:, m : m + 1], xch,
                        op0=ADD, op1=ADD,
                    )
                    nc.gpsimd.tensor_scalar_max(dst, u[:], 0.0)

            def emit_conv3_m(s, m, ofull):
                emit_conv3_half(s, m, 0, ofull, True)
                emit_conv3_half(s, m, 1, ofull, False)
                nc.sync.dma_start(out_d[s, m], ofull[:, m * P : (m + 1) * P])

            emit_router(0)
            alloc_w(0)
            emit_combine_chunk(0, 0)

            # ============ per-sample convs (combine pipelined ahead) ========
            out2_l = {}
            for s in range(BS):
                xs = xs_l[s]
                w1c, w2c, w3c = w_l[s]

                # ---- conv1 (1x1) + bn1 + relu -> padded mid1 [128, 30, 30] ----
                mid1 = mid1s[s % 2]
                for c in range(2):
                    ps1 = psc12p.tile([128, 14, 28], F32, tag="convps")
                    for k in range(4):
                        mm(
                            ps1[:],
                            w1c[:, k * 128 : (k + 1) * 128],
                            xs[k][:, c * NCH : (c + 1) * NCH],
                            start=(k == 0),
                            stop=(k == 3),
                        )
                    nc.scalar.activation(
                        mid1[:, 14 * c + 1 : 14 * c + 15, 1:29],
                        ps1[:],
                        Relu,
                        bias=bias1[:],
                    )

                if s == 0:
                    # stagger sample 0's remaining combine chunks behind the
                    # bank DMAs instead of head-blocking the PE in the prologue
                    emit_combine_chunk(0, 1)
                    emit_combine_chunk(0, 2)
                    emit_combine_chunk(0, 3)

                # ---- conv2 (3x3, pad 1) + bn2 + relu -> out2 [128, 784] ----
                out2 = actp.tile([128, P], F16, tag="out2")
                out2_l[s] = out2
                last = s == BS - 1
                if last:
                    ofull_t = actp.tile([128, 4 * P], F16, tag="ofull",
                                        name="ofull_last")
                for c in range(2):
                    ps2 = psc12p.tile([128, 14, 28], F32, tag="convps")
                    idx = 0
                    for dy in range(3):
                        for dx in range(3):
                            mm(
                                ps2[:],
                                w2c[:, (dy * 3 + dx) * 128 : (dy * 3 + dx + 1) * 128],
                                mid1[:, 14 * c + dy : 14 * c + dy + 14, dx : dx + 28],
                                start=(idx == 0),
                                stop=(idx == 8),
                            )
                            idx += 1
                    nc.scalar.activation(
                        out2[:, c * NCH : (c + 1) * NCH], ps2[:], Relu, bias=bias2[:]
                    )
                    if last:
                        # drain the last sample's conv3 halves as early as
                        # possible: the kernel end is gated by these drains
                        # and by DMA issue time, not by the PE -> spread the
                        # 8 half-DMAs across two descriptor queues
                        for m in range(4):
                            emit_conv3_half(s, m, c, ofull_t, (m + c) % 2 == 0)
                            q = (nc.sync, nc.scalar, nc.gpsimd, nc.scalar)[m]
                            q.dma_start(
                                out_d[s, m][:, c * NCH : (c + 1) * NCH],
                                ofull_t[:, m * P + c * NCH : m * P + (c + 1) * NCH],
                            )

                if s == 0:
                    emit_combine_chunk(0, 4)
                    emit_router(1)
                elif s + 2 < BS:
                    emit_router(s + 2)

                # ---- conv3 + bn3 + residual + relu, interleaved with the
                # next sample's combine so psum drains keep pace with the PE --
                if s + 1 < BS:
                    ofull = actp.tile([128, 4 * P], F16, tag="ofull")
                    alloc_w(s + 1)
                    if s == 0:
                        # give ids8(1) (generated just above) a head start
                        emit_conv3_m(0, 0, ofull)
                        emit_conv3_m(0, 1, ofull)
                        emit_combine_chunk(1, 0)
                        emit_conv3_m(0, 2, ofull)
                        emit_combine_chunk(1, 1)
                        emit_conv3_m(0, 3, ofull)
                        emit_combine_chunk(1, 2)
                        emit_combine_chunk(1, 3)
                        emit_combine_chunk(1, 4)
                        emit_router(2)
                    else:
                        emit_combine_chunk(s + 1, 0)
                        emit_conv3_m(s, 0, ofull)
                        emit_combine_chunk(s + 1, 1)
                        emit_conv3_m(s, 1, ofull)
                        emit_combine_chunk(s + 1, 2)
                        emit_conv3_m(s, 2, ofull)
                        emit_combine_chunk(s + 1, 3)
                        emit_combine_chunk(s + 1, 4)
                        emit_conv3_m(s, 3, ofull)

    nc.compile()
    return nc


_NC_CACHE = None


def _get_program():
    global _NC_CACHE
    if _NC_CACHE is None:
        _NC_CACHE = build_program()
    return _NC_CACHE


def prepare_inputs(
    x, router_w, router_b, w1, w2, w3,
    g1, b1, m1, v1, g2, b2, m2, v2, g3, b3, m3, v3,
):
    """Host-side preprocessing -> per-core in_maps."""
    f = np.float32
    x = np.asarray(x, f)
    router_w = np.asarray(router_w, f)
    router_b = np.asarray(router_b, f)
    w1 = np.asarray(w1, f)
    w2 = np.asarray(w2, f)
    w3 = np.asarray(w3, f)

    s1 = np.asarray(g1, f) / np.sqrt(np.asarray(v1, f) + EPS)
    s2 = np.asarray(g2, f) / np.sqrt(np.asarray(v2, f) + EPS)
    s3 = np.asarray(g3, f) / np.sqrt(np.asarray(v3, f) + EPS)
    bb1 = np.asarray(b1, f) - np.asarray(m1, f) * s1
    bb2 = np.asarray(b2, f) - np.asarray(m2, f) * s2
    bb3 = np.asarray(b3, f) - np.asarray(m3, f) * s3

    # bank1: [E, o=128, i=512] * s1[o] -> rows i%128, cols (e, it, o)
    w1s = w1[:, :, :, 0, 0] * s1[None, :, None]
    bank1 = np.ascontiguousarray(
        w1s.transpose(0, 2, 1).reshape(E, 4, 128, 128).transpose(2, 0, 1, 3)
        .reshape(128, E * 512)
    )
    # bank2: [E, o, ci, dy, dx] * s2[o] -> rows ci, cols (e, tap, o)
    w2s = w2 * s2[None, :, None, None, None]
    b2flat = (
        w2s.transpose(0, 3, 4, 2, 1).reshape(E, 9, 128, 128).transpose(2, 0, 1, 3)
        .reshape(128, E, 1152)
    )
    # bank3: [E, o=512, ci=128] * s3[o] -> rows ci, cols (e, m, o)
    w3s = w3[:, :, :, 0, 0] * s3[None, :, None]
    bank3 = np.ascontiguousarray(
        w3s.transpose(0, 2, 1).transpose(1, 0, 2).reshape(128, E * 512)
    )

    # per-chunk [128, E, wid] views, expert-major delta banks (x0.5) in fp8
    chunks = {
        "db1": bank1.reshape(128, E, 512),
        "db2a": b2flat[:, :, 0:512],
        "db2b": b2flat[:, :, 512:1024],
        "db2c": b2flat[:, :, 1024:1152],
        "db3": bank3.reshape(128, E, 512),
    }
    dbanks = {
        k: np.ascontiguousarray((0.5 * v).reshape(128, -1)).astype(NP8)
        for k, v in chunks.items()
    }
    # mean term: hi fp8 + (residual*16) fp8, chunk-ordered
    mean_parts = []
    for k in ("db1", "db2a", "db2b", "db2c", "db3"):
        M = 0.5 * chunks[k].sum(axis=1)          # [128, wid]
        hi = M.astype(NP8)
        lo = ((M - hi.astype(f)) * 16.0).astype(NP8)
        mean_parts += [hi, lo]
    dmean = np.concatenate(mean_parts, axis=1)
    assert dmean.shape == (128, MEANW)

    idpair = np.zeros((128, 256), NP8)
    idpair[:, 0:128] = np.eye(128, dtype=NP8)
    idpair[:, 128:256] = (np.eye(128, dtype=f) / 16.0).astype(NP8)

    rwt = np.ascontiguousarray(
        (router_w / float(P)).T.reshape(4, 128, E)
    ).astype(np.float16)
    cc = np.zeros((128, 169), np.float16)
    cc[:, 0:32] = rwt.transpose(1, 0, 2).reshape(128, 32)
    cc[:, 32:160] = np.eye(128, dtype=np.float16)
    cc[:, 160] = 1.0
    cc[0, 161:169] = router_b.astype(np.float16)
    biasp = np.zeros((128, 6), f)
    biasp[:, 0] = bb1
    biasp[:, 1] = bb2
    biasp[:, 2:6] = bb3.reshape(4, 128).T

    x16 = x.reshape(B, 4, 128, P).astype(np.float16)

    shared = {
        **dbanks,
        "dmean": dmean,
        "idpair": idpair,
        "cc": cc,
        "biasp": biasp,
    }
    in_maps = []
    for c in range(NCORES):
        m = dict(shared)
        m["x"] = np.ascontiguousarray(x16[c * BS : (c + 1) * BS])
        in_maps.append(m)
    return in_maps


def run(in_maps, trace=False, tmpdir=None):
    nc = _get_program()
    res = bass_utils.run_bass_kernel_spmd(
        nc, in_maps, core_ids=list(range(NCORES)), trace=trace, tmpdir=tmpdir
    )
    outs = [np.asarray(r["out"], np.float32) for r in res.results]
    full = np.concatenate(outs, axis=0).reshape(B, CIN, H, H)
    return full, res


def kernel(**inputs):
    in_maps = prepare_inputs(**inputs)
    full, _ = run(in_maps, trace=False)
    return full


# revision 38
# speedup vs baseline: 1.0102x; 1.0028x over previous
"""
Bass/Trainium2 kernel for nn_BottleneckShared (moe_routing).

Computation (per sample b):
    rw   = sigmoid(mean_hw(x) @ router_w.T + router_b)          # [E]
    Wk_b = sum_e rw[e] * wk[e]            (k = 1,2,3)           # per-sample conv kernels
    out  = relu(bn3(conv3(relu(bn2(conv2(relu(bn1(conv1(x)))))))) + x)

Sharding: data-parallel over batch. 64 samples -> 8 NeuronCores x 8 samples.

Key device-side design (v2, PE-minimal):
 - PE matmul cost = out-free-size x cycles/row; fp8e4+DoubleRow runs at
   0.5 cycles/row with TWO independent 128-contraction planes per
   instruction (4x fp16 throughput).
 - The per-sample weight combine exploits sigmoid(z) = 0.5 + 0.5*tanh(z/2):
       W_c = 0.5*sum_e W_e  +  sum_e (0.5*tanh(z_e/2)) * W_e
   The mean term is sample-independent: host precomputes it as an fp8
   hi + (lo*16) pair (combined on PE with an [I | I/16] fp8 lhsT, one
   DoubleRow matmul). The delta term is ~2% of the mean, so single fp8
   precision suffices: expert pairs go in one DoubleRow matmul each with
   lhsT = [tanh_e*I | tanh_e'*I] (fp8, generated per-sample on DVE/Pool).
   Per 512-col chunk: 5 DoubleRow matmuls vs 8 fp16 matmuls -> 3.2x.
 - Router pooling runs on the PE (28 ap_size=8 matmuls: lhsT = x pixel
   chunks, rhs = router weights; psum-accumulated), then a ones-broadcast
   matmul reduces the 112 partial partitions and broadcasts z to all 128
   partitions; ACT applies tanh(z/2) via activation scale=0.5.
 - Convs stay fp16 (x and activations need >fp8 precision; precision
   hi/lo splitting costs as much as fp16).
 - Residual + relu run off-PE: conv3 psum drains via ACT Identity(+bn3
   bias) or DVE STT(+bias+x), then DVE tensor_scalar max(.,0) -> fp16 out.
 - All PSUM->SBUF drains are spread across ACT/DVE/Pool so every
   non-tensor engine stays below the PE's ~7.9us/sample.
"""

import sys

import ml_dtypes
import numpy as np

sys.path.insert(0, "/opt/trn_rl_repo")

import concourse.bacc as bacc
import concourse.bass as bass
import concourse.mybir as mybir
import concourse.tile as tile
from concourse import bass_utils

EPS = 1e-5

B = 64          # global batch
NCORES = 8
BS = B // NCORES  # samples per core
E = 8           # experts
CIN = 512
WID = 128       # bottleneck width
COUT = 512
H = 28
P = H * H       # 784 pixels
NCH = 392       # pixels per conv output chunk (14 rows)
PC = 112        # router pixel chunk (784 = 7*112)

F8 = mybir.dt.float8e4
F16 = mybir.dt.float16
F32 = mybir.dt.float32
NP8 = ml_dtypes.float8_e4m3

# combine chunks: (dst, dst_off, width, bank_name, mean_off)
CHUNKS = [
    ("w1c", 0, 512, "db1", 0),
    ("w2c", 0, 512, "db2a", 1024),
    ("w2c", 512, 512, "db2b", 2048),
    ("w2c", 1024, 128, "db2c", 3072),
    ("w3c", 0, 512, "db3", 3328),
]
MEANW = 4352  # 2 * 2176


def _r2(ap):
    return ap.rearrange("p (two n) -> p two n", two=2)


def build_program():
    nc = bacc.Bacc("TRN2", target_bir_lowering=False, debug=False)

    # ---- DRAM I/O (per-core shapes) ----
    x_d = nc.dram_tensor("x", [BS, 4, 128, P], F16, kind="ExternalInput")
    db1_d = nc.dram_tensor("db1", [128, E * 512], F8, kind="ExternalInput")
    db2a_d = nc.dram_tensor("db2a", [128, E * 512], F8, kind="ExternalInput")
    db2b_d = nc.dram_tensor("db2b", [128, E * 512], F8, kind="ExternalInput")
    db2c_d = nc.dram_tensor("db2c", [128, E * 128], F8, kind="ExternalInput")
    db3_d = nc.dram_tensor("db3", [128, E * 512], F8, kind="ExternalInput")
    dmean_d = nc.dram_tensor("dmean", [128, MEANW], F8, kind="ExternalInput")
    idpair_d = nc.dram_tensor("idpair", [128, 256], F8, kind="ExternalInput")
    cc_d = nc.dram_tensor("cc", [128, 169], F16, kind="ExternalInput")
    biasp_d = nc.dram_tensor("biasp", [128, 6], F32, kind="ExternalInput")
    out_d = nc.dram_tensor("out", [BS, 4, 128, P], F16, kind="ExternalOutput")

    Relu = mybir.ActivationFunctionType.Relu
    Tanh = mybir.ActivationFunctionType.Tanh
    Ident = mybir.ActivationFunctionType.Identity
    Copy = mybir.ActivationFunctionType.Copy
    ADD = mybir.AluOpType.add
    DR = mybir.MatmulPerfMode.DoubleRow
    mm = nc.tensor.matmul

    with tile.TileContext(nc) as tc:
        with (
            tc.tile_pool(name="const", bufs=1) as constp,
            tc.tile_pool(name="xin", bufs=6) as xp,
            tc.tile_pool(name="xsplit", bufs=1) as xsp,
            tc.tile_pool(name="ids", bufs=3) as idsp,
            tc.tile_pool(name="comb", bufs=2) as combp,
            tc.tile_pool(name="act", bufs=2) as actp,
            tc.tile_pool(name="small", bufs=3) as smallp,
            tc.tile_pool(name="t3", bufs=4) as t3p,
            tc.tile_pool(name="rwb", bufs=3) as rwbp,
            tc.tile_pool(name="pscomb", bufs=2, space=bass.MemorySpace.PSUM) as pscombp,
            tc.tile_pool(name="psc12", bufs=3, space=bass.MemorySpace.PSUM) as psc12p,
            tc.tile_pool(name="psc3", bufs=2, space=bass.MemorySpace.PSUM) as psc3p,
            tc.tile_pool(name="psr", bufs=1, space=bass.MemorySpace.PSUM) as psrp,
        ):
            # ---- persistent constants ----
            db1 = constp.tile([128, E * 512], F8)
            db2a = constp.tile([128, E * 512], F8)
            db2b = constp.tile([128, E * 512], F8)
            db2c = constp.tile([128, E * 128], F8)
            db3 = constp.tile([128, E * 512], F8)
            dmean = constp.tile([128, MEANW], F8)
            idpair = constp.tile([128, 256], F8)
            cc = constp.tile([128, 169], F16)
            rwt = cc[:, 0:32]
            ident = cc[:, 32:160]
            onescol = cc[:, 160:161]
            rbrow = cc[0:1, 161:169]
            biasp = constp.tile([128, 6], F32)
            bias1 = biasp[:, 0:1]
            bias2 = biasp[:, 1:2]
            bias3 = biasp[:, 2:6]
            # persistent padded conv2 inputs; border zeroed once
            mid1s = [
                constp.tile([128, 30, 30], F16, name=f"mid1_{i}") for i in range(2)
            ]

            banks = {"db1": db1, "db2a": db2a, "db2b": db2b, "db2c": db2c, "db3": db3}

            # DMA issue order = device service order.
            xs_l, ids_l, tanh_l = [], [], []
            for s in range(BS):
                big = xp.tile([128, 4 * P], F16, tag="xs", name=f"xs{s}")
                xs_l.append([big[:, t * P : (t + 1) * P] for t in range(4)])

            def load_x(s):
                nc.sync.dma_start(
                    xs_l[s][0].tensor[:, :], x_d[s].transpose([1, 0, 2])
                )

            nc.sync.dma_start(cc[:], cc_d[:])
            # x0 split in halves so the router can start on tiles 0-1 early
            x0t = xs_l[0][0].tensor
            nc.sync.dma_start(
                x0t[:, : 2 * P], x_d[0, 0:2].transpose([1, 0, 2])
            )
            nc.sync.dma_start(
                x0t[:, 2 * P :], x_d[0, 2:4].transpose([1, 0, 2])
            )
            nc.sync.dma_start(idpair[:], idpair_d[:])
            nc.sync.dma_start(dmean[:, :2048], dmean_d[:, :2048])
            nc.sync.dma_start(db1[:], db1_d[:])
            nc.sync.dma_start(biasp[:], biasp_d[:])
            nc.sync.dma_start(db2a[:], db2a_d[:])
            load_x(1)
            nc.sync.dma_start(db2b[:], db2b_d[:])
            nc.sync.dma_start(dmean[:, 2048:], dmean_d[:, 2048:])
            nc.sync.dma_start(db2c[:], db2c_d[:])
            nc.sync.dma_start(db3[:], db3_d[:])
            for s in range(2, BS):
                load_x(s)

            for m1 in mid1s:
                nc.gpsimd.memset(m1[:], 0.0)

            # ================= router ======================================
            def emit_router(s):
                xs = xs_l[s]
                psr1 = psrp.tile([128, 8], F32, tag="psr", name=f"psr1_{s}")
                idx = 0
                for t in range(4):
                    for pc in range(7):
                        mm(
                            psr1[:PC, :],
                            xs[t][:, pc * PC : (pc + 1) * PC],
                            rwt[:, t * 8 : (t + 1) * 8],
                            start=(idx == 0),
                            stop=(idx == 27),
                        )
                        idx += 1
                r1 = smallp.tile([PC, 8], F16, tag="r1")
                nc.vector.tensor_copy(r1[:], psr1[:PC, :])
                psr2 = psrp.tile([128, 8], F32, tag="psr", name=f"psr2_{s}")
                mm(
                    psr2[:],
                    onescol[:PC, 0:1].broadcast_to([PC, 128]),
                    r1[:],
                    start=True,
                    stop=False,
                )
                mm(
                    psr2[:],
                    onescol[0:1, 0:1].broadcast_to([1, 128]),
                    rbrow,
                    start=False,
                    stop=True,
                )
                tanhv = rwbp.tile([128, 8], F32, tag="tanhv", name=f"tanhv{s}")
                nc.scalar.activation(tanhv[:], psr2[:], Tanh, scale=0.5)
                tanh_l.append(tanhv)
                ids8 = idsp.tile([128, E * 128], F8, tag="ids8", name=f"ids8{s}")
                for e in range(E):
                    eng = nc.vector if e % 2 == 0 else nc.gpsimd
                    eng.tensor_scalar_mul(
                        ids8[:, e * 128 : (e + 1) * 128], ident, tanhv[:, e : e + 1]
                    )
                ids_l.append(ids8)

            # ---- combine all chunks on PE (fp8 DoubleRow), 1 sample ahead ----
            w_l = []

            def alloc_w(s):
                w1c = combp.tile([128, 512], F16, tag="w1c", name=f"w1c{s}")
                w2c = combp.tile([128, 1152], F16, tag="w2c", name=f"w2c{s}")
                w3c = combp.tile([128, 512], F16, tag="w3c", name=f"w3c{s}")
                w_l.append((w1c, w2c, w3c))

            def emit_combine_chunk(s, i):
                ids8 = ids_l[s]
                w1c, w2c, w3c = w_l[s]
                dsts = {"w1c": w1c, "w2c": w2c, "w3c": w3c}
                dname, d0, wid, bname, moff = CHUNKS[i]
                bank = banks[bname]
                psc = pscombp.tile([128, 512], F32, tag="psc")
                mm(
                    psc[:, :wid],
                    _r2(idpair[:]),
                    _r2(dmean[:, moff : moff + 2 * wid]),
                    start=True,
                    stop=False,
                    perf_mode=DR,
                )
                for j in range(4):
                    mm(
                        psc[:, :wid],
                        _r2(ids8[:, j * 256 : (j + 1) * 256]),
                        _r2(bank[:, j * 2 * wid : (j + 1) * 2 * wid]),
                        start=False,
                        stop=(j == 3),
                        perf_mode=DR,
                    )
                dst = dsts[dname][:, d0 : d0 + wid]
                # drains: GPSIMD cannot read PSUM -> alternate DVE / ACT
                if i in (0, 2):
                    nc.vector.tensor_scalar_add(dst, psc[:, :wid], 0.0)
                else:
                    nc.scalar.activation(dst, psc[:, :wid], Copy)

            def emit_conv3_half(s, m, c, ofull, path_a):
                xs = xs_l[s]
                w3c = w_l[s][2]
                out2 = out2_l[s]
                ps3 = psc3p.tile([128, 14, 28], F32, tag="c3ps")
                mm(
                    ps3[:],
                    w3c[:, m * 128 : (m + 1) * 128],
                    out2[:, c * NCH : (c + 1) * NCH],
                    start=True,
                    stop=True,
                )
                ps3f = ps3[:].rearrange("p a b -> p (a b)")
                xch = xs[m][:, c * NCH : (c + 1) * NCH]
                dst = ofull[:, m * P + c * NCH : m * P + (c + 1) * NCH]
                u = t3p.tile([128, NCH], F16, tag="u3")
                if path_a:
                    # ACT drains bn3+bias, DVE adds residual, Pool relus
                    t = t3p.tile([128, NCH], F16, tag="t3")
                    nc.scalar.activation(
                        t[:], ps3f, Ident, bias=bias3[:, m : m + 1]
                    )
                    nc.vector.tensor_tensor(u[:], t[:], xch, op=ADD)
                    nc.vector.tensor_scalar_max(dst, u[:], 0.0)
                else:
                    # single DVE pass: (psum + bias3) + x; relu on Pool
                    nc.vector.scalar_tensor_tensor(
                        u[:], ps3f, bias3[:, m : m + 1], xch,
                        op0=ADD, op1=ADD,
                    )
                    nc.gpsimd.tensor_scalar_max(dst, u[:], 0.0)

            def emit_conv3_m(s, m, ofull):
                emit_conv3_half(s, m, 0, ofull, True)
                emit_conv3_half(s, m, 1, ofull, False)
                nc.sync.dma_start(out_d[s, m], ofull[:, m * P : (m + 1) * P])

            emit_router(0)
            alloc_w(0)
            emit_combine_chunk(0, 0)

            # ============ per-sample convs (combine pipelined ahead) ========
            out2_l = {}
            for s in range(BS):
                xs = xs_l[s]
                w1c, w2c, w3c = w_l[s]

                # ---- conv1 (1x1) + bn1 + relu -> padded mid1 [128, 30, 30] ----
                mid1 = mid1s[s % 2]
                for c in range(2):
                    ps1 = psc12p.tile([128, 14, 28], F32, tag="convps")
                    for k in range(4):
                        mm(
                            ps1[:],
                            w1c[:, k * 128 : (k + 1) * 128],
                            xs[k][:, c * NCH : (c + 1) * NCH],
                            start=(k == 0),
                            stop=(k == 3),
                        )
                    nc.scalar.activation(
                        mid1[:, 14 * c + 1 : 14 * c + 15, 1:29],
                        ps1[:],
                        Relu,
                        bias=bias1[:],
                    )

                if s == 0:
                    # stagger sample 0's remaining combine chunks behind the
                    # bank DMAs instead of head-blocking the PE in the prologue
                    emit_combine_chunk(0, 1)
                    emit_combine_chunk(0, 2)
                    emit_combine_chunk(0, 3)

                # ---- conv2 (3x3, pad 1) + bn2 + relu -> out2 [128, 784] ----
                out2 = actp.tile([128, P], F16, tag="out2")
                out2_l[s] = out2
                last = s == BS - 1
                if last:
                    ofull_t = actp.tile([128, 4 * P], F16, tag="ofull",
                                        name="ofull_last")
                for c in range(2):
                    ps2 = psc12p.tile([128, 14, 28], F32, tag="convps")
                    idx = 0
                    for dy in range(3):
                        for dx in range(3):
                            mm(
                                ps2[:],
                                w2c[:, (dy * 3 + dx) * 128 : (dy * 3 + dx + 1) * 128],
                                mid1[:, 14 * c + dy : 14 * c + dy + 14, dx : dx + 28],
                                start=(idx == 0),
                                stop=(idx == 8),
                            )
                            idx += 1
                    nc.scalar.activation(
                        out2[:, c * NCH : (c + 1) * NCH], ps2[:], Relu, bias=bias2[:]
                    )
                    if last:
                        # drain the last sample's conv3 halves as early as
                        # possible: the kernel end is gated by these drains
                        # and by DMA issue time, not by the PE -> spread the
                        # 8 half-DMAs across two descriptor queues
                        for m in range(4):
                            emit_conv3_half(s, m, c, ofull_t, (m + c) % 2 == 0)
                            q = nc.sync if m % 2 == 0 else nc.scalar
                            q.dma_start(
                                out_d[s, m][:, c * NCH : (c + 1) * NCH],
                                ofull_t[:, m * P + c * NCH : m * P + (c + 1) * NCH],
                            )

                if s == 0:
                    emit_combine_chunk(0, 4)
                    emit_router(1)
                elif s + 2 < BS:
                    emit_router(s + 2)

                # ---- conv3 + bn3 + residual + relu, interleaved with the
                # next sample's combine so psum drains keep pace with the PE --
                if s + 1 < BS:
                    ofull = actp.tile([128, 4 * P], F16, tag="ofull")
                    alloc_w(s + 1)
                    if s == 0:
                        # give ids8(1) (generated just above) a head start
                        emit_conv3_m(0, 0, ofull)
                        emit_conv3_m(0, 1, ofull)
                        emit_combine_chunk(1, 0)
                        emit_conv3_m(0, 2, ofull)
                        emit_combine_chunk(1, 1)
                        emit_conv3_m(0, 3, ofull)
                        emit_combine_chunk(1, 2)
                        emit_combine_chunk(1, 3)
                        emit_combine_chunk(1, 4)
                        emit_router(2)
                    else:
                        emit_combine_chunk(s + 1, 0)
                        emit_conv3_m(s, 0, ofull)
                        emit_combine_chunk(s + 1, 1)
                        emit_conv3_m(s, 1, ofull)
                        emit_combine_chunk(s + 1, 2)
                        emit_conv3_m(s, 2, ofull)
                        emit_combine_chunk(s + 1, 3)
                        emit_combine_chunk(s + 1, 4)
                        emit_conv3_m(s, 3, ofull)

    nc.compile()
    return nc


_NC_CACHE = None


def _get_program():
    global _NC_CACHE
    if _NC_CACHE is None:
        _NC_CACHE = build_program()
    return _NC_CACHE


def prepare_inputs(
    x, router_w, router_b, w1, w2, w3,
    g1, b1, m1, v1, g2, b2, m2, v2, g3, b3, m3, v3,
):
    """Host-side preprocessing -> per-core in_maps."""
    f = np.float32
    x = np.asarray(x, f)
    router_w = np.asarray(router_w, f)
    router_b = np.asarray(router_b, f)
    w1 = np.asarray(w1, f)
    w2 = np.asarray(w2, f)
    w3 = np.asarray(w3, f)

    s1 = np.asarray(g1, f) / np.sqrt(np.asarray(v1, f) + EPS)
    s2 = np.asarray(g2, f) / np.sqrt(np.asarray(v2, f) + EPS)
    s3 = np.asarray(g3, f) / np.sqrt(np.asarray(v3, f) + EPS)
    bb1 = np.asarray(b1, f) - np.asarray(m1, f) * s1
    bb2 = np.asarray(b2, f) - np.asarray(m2, f) * s2
    bb3 = np.asarray(b3, f) - np.asarray(m3, f) * s3

    # bank1: [E, o=128, i=512] * s1[o] -> rows i%128, cols (e, it, o)
    w1s = w1[:, :, :, 0, 0] * s1[None, :, None]
    bank1 = np.ascontiguousarray(
        w1s.transpose(0, 2, 1).reshape(E, 4, 128, 128).transpose(2, 0, 1, 3)
        .reshape(128, E * 512)
    )
    # bank2: [E, o, ci, dy, dx] * s2[o] -> rows ci, cols (e, tap, o)
    w2s = w2 * s2[None, :, None, None, None]
    b2flat = (
        w2s.transpose(0, 3, 4, 2, 1).reshape(E, 9, 128, 128).transpose(2, 0, 1, 3)
        .reshape(128, E, 1152)
    )
    # bank3: [E, o=512, ci=128] * s3[o] -> rows ci, cols (e, m, o)
    w3s = w3[:, :, :, 0, 0] * s3[None, :, None]
    bank3 = np.ascontiguousarray(
        w3s.transpose(0, 2, 1).transpose(1, 0, 2).reshape(128, E * 512)
    )

    # per-chunk [128, E, wid] views, expert-major delta banks (x0.5) in fp8
    chunks = {
        "db1": bank1.reshape(128, E, 512),
        "db2a": b2flat[:, :, 0:512],
        "db2b": b2flat[:, :, 512:1024],
        "db2c": b2flat[:, :, 1024:1152],
        "db3": bank3.reshape(128, E, 512),
    }
    dbanks = {
        k: np.ascontiguousarray((0.5 * v).reshape(128, -1)).astype(NP8)
        for k, v in chunks.items()
    }
    # mean term: hi fp8 + (residual*16) fp8, chunk-ordered
    mean_parts = []
    for k in ("db1", "db2a", "db2b", "db2c", "db3"):
        M = 0.5 * chunks[k].sum(axis=1)          # [128, wid]
        hi = M.astype(NP8)
        lo = ((M - hi.astype(f)) * 16.0).astype(NP8)
        mean_parts += [hi, lo]
    dmean = np.concatenate(mean_parts, axis=1)
    assert dmean.shape == (128, MEANW)

    idpair = np.zeros((128, 256), NP8)
    idpair[:, 0:128] = np.eye(128, dtype=NP8)
    idpair[:, 128:256] = (np.eye(128, dtype=f) / 16.0).astype(NP8)

    rwt = np.ascontiguousarray(
        (router_w / float(P)).T.reshape(4, 128, E)
    ).astype(np.float16)
    cc = np.zeros((128, 169), np.float16)
    cc[:, 0:32] = rwt.transpose(1, 0, 2).reshape(128, 32)
    cc[:, 32:160] = np.eye(128, dtype=np.float16)
    cc[:, 160] = 1.0
    cc[0, 161:169] = router_b.astype(np.float16)
    biasp = np.zeros((128, 6), f)
    biasp[:, 0] = bb1
    biasp[:, 1] = bb2
    biasp[:, 2:6] = bb3.reshape(4, 128).T

    x16 = x.reshape(B, 4, 128, P).astype(np.float16)

    shared = {
        **dbanks,
        "dmean": dmean,
        "idpair": idpair,
        "cc": cc,
        "biasp": biasp,
    }
    in_maps = []
    for c in range(NCORES):
        m = dict(shared)
        m["x"] = np.ascontiguousarray(x16[c * BS : (c + 1) * BS])
        in_maps.append(m)
    return in_maps


def run(in_maps, trace=False, tmpdir=None):
    nc = _get_program()
    res = bass_utils.run_bass_kernel_spmd(
        nc, in_maps, core_ids=list(range(NCORES)), trace=trace, tmpdir=tmpdir
    )
    outs = [np.asarray(r["out"], np.float32) for r in res.results]
    full = np.concatenate(outs, axis=0).reshape(B, CIN, H, H)
    return full, res


def kernel(**inputs):
    in_maps = prepare_inputs(**inputs)
    full, _ = run(in_maps, trace=False)
    return full
